# revision 25
# baseline (speedup 1.0000x reference)
"""Trainium2 Bass kernel for nn_LogicReasoningEncoder (GNN message passing).

Sharding: 8 cores = 4 batches x 2 target-node halves. Each core owns the
edges whose target node falls in its half, bucketed into 8 fixed-capacity
node blocks of 128 nodes so that every core runs the identical SPMD program.
Scatter-softmax is reformulated without the max pass (att is leaky-relu
bounded, so exp never overflows) and the alpha normalization is deferred to
a per-node divide after aggregation, so cross-core work is just one
pair-wise AllGather of updated node features per layer.

v2 layout notes:
- conf_embeds arrives from the host already transposed to feature-major
  [D, ecap] bf16, so no on-device transposes are needed.
- All per-edge scalar rows (att_rel, beta, den_lin, att) are computed
  directly in "chunk layout" ([128 edges-within-chunk, chunks]) via
  per-chunk matmuls, eliminating the DRAM row->chunk round trips.
- The scatter one-hot matrix is loaded into SBUF once and reused by all
  three layers.
- rel_table and h live in SBUF and are gathered with SBUF-source
  dma_gather (no HBM small-descriptor penalty).
"""

import os
import sys
import numpy as np

for _p in ("/opt/trn_rl_repo", "/root/.axon_site/_ro/trn_rl_repo"):
    if _p not in sys.path:
        sys.path.append(_p)

# debug switch: "sbuf" gathers from SBUF-resident tables, "hbm" from DRAM
GATHER_SRC = os.environ.get("KGATHER", "sbuf")
# NOTE: SWDGE queue 1 corrupts gathers on HW (verified empirically); use queue 0.
GQ = int(os.environ.get("KQUEUE", "0"))  # 2 = alternate queues, else fixed queue id

import concourse.bass as bass
import concourse.mybir as mybir
from concourse import bacc, tile
from concourse.bass_utils import run_bass_kernel_spmd

F32 = mybir.dt.float32
BF16 = mybir.dt.bfloat16
I16 = mybir.dt.int16
BF16_NP = mybir.dt.np(BF16)

B, N, E, D, L, NREL, TAU = 4, 2048, 32768, 128, 3, 1000, 0.1
NHALF = N // 2          # nodes per core
NBLK = NHALF // 128     # 8 node blocks per core
P = 128
NRELP = 1024            # rel table padded to 8 tokens x 128 ranks


# ----------------------------------------------------------------------------
# Host-side sharding / layout prep (index manipulation + layout only)
# ----------------------------------------------------------------------------

def _chunkify(x, ecap):
    """[ecap] -> [128, ecap//128] with x[c*128+p] at [p, c]."""
    return np.ascontiguousarray(x.reshape(ecap // 128, 128).T)


def _wrap16(x, ecap):
    """[ecap] -> int16 [128, ecap//16]: x[i] at [i%16, i//16], tiled x8 for Q7 cores."""
    w = np.ascontiguousarray(x.astype(np.int16).reshape(ecap // 16, 16).T)
    return np.ascontiguousarray(np.tile(w, (8, 1)))


def prepare_core_inputs(inputs):
    ei = np.asarray(inputs["edge_index"])          # [B, 2, E] int32
    rels = np.asarray(inputs["rels"])              # [B, E]
    scores = np.asarray(inputs["scores"])          # [B, E] f32
    cm = np.asarray(inputs["edge_conf_mask"])      # [B, E] bool
    em = np.asarray(inputs["edge_mask"])           # [B, E] bool
    conf = np.asarray(inputs["conf_embeds"])       # [B, E, D] f32

    # Fixed per-block edge capacity, uniform across all cores (SPMD).
    max_cnt = 0
    per_core = []
    for core in range(8):
        b, half = core // 2, core % 2
        base = half * NHALF
        tgt = ei[b, 1]
        sel = np.nonzero((tgt >= base) & (tgt < base + NHALF))[0]
        ltgt = tgt[sel] - base
        blk = ltgt >> 7
        cnts = np.bincount(blk, minlength=NBLK)
        max_cnt = max(max_cnt, int(cnts.max()))
        per_core.append((b, base, sel, ltgt, blk))

    e_blk = ((max_cnt + 383) // 384) * 384
    ecap = NBLK * e_blk

    h0 = np.zeros((N, D), dtype=BF16_NP)
    h0[0, :] = 1.0

    rel_pad = np.zeros((NRELP, D), np.float32)
    rel_pad[:NREL] = np.asarray(inputs["rel_table"])
    rel_bf = rel_pad.astype(BF16_NP)

    in_maps = []
    for core in range(8):
        b, base, sel, ltgt, blk = per_core[core]
        order = np.argsort(blk, kind="stable")
        perm = sel[order]                 # original edge ids, block-sorted
        lt = ltgt[order]
        bk = blk[order]
        cnts = np.bincount(bk, minlength=NBLK)
        slot = np.concatenate(
            [bb * e_blk + np.arange(cnts[bb]) for bb in range(NBLK)]
        ).astype(np.int64) if len(perm) else np.zeros(0, np.int64)

        src_p = np.zeros(ecap, np.int32)
        rels_p = np.zeros(ecap, np.int32)
        scores_p = np.zeros(ecap, np.float32)
        em_p = np.zeros(ecap, np.float32)
        cm_p = np.zeros(ecap, np.float32)
        conf_p = np.zeros((ecap, D), np.float32)
        oh = np.zeros((ecap, P), np.float32)

        src_p[slot] = ei[b, 0][perm]
        rels_p[slot] = rels[b][perm]
        scores_p[slot] = scores[b][perm]
        em_p[slot] = em[b][perm].astype(np.float32)
        cm_p[slot] = cm[b][perm].astype(np.float32)
        conf_p[slot] = conf[b][perm]
        j = lt - (slot // e_blk) * 128
        oh[slot, j] = 1.0
        # partition-major one-hot: [p, c*128 + j]
        oh_pm = np.ascontiguousarray(
            oh.reshape(ecap // 128, 128, 128).transpose(1, 0, 2).reshape(P, ecap)
        ).astype(BF16_NP)

        hown = np.zeros((P, NBLK * 128), np.float32)
        if core % 2 == 0:
            hown[0, 0:128] = 1.0  # node 0 lives at block 0, partition 0

        m = {
            "conf_fm": np.ascontiguousarray(conf_p.T).astype(BF16_NP),
            "onehot_pm": oh_pm,
            "srcz_row": (src_p == 0).astype(BF16_NP),
            "scores_ch": _chunkify(scores_p, ecap),
            "em_ch": _chunkify(em_p, ecap),
            "cm_ch": _chunkify(cm_p, ecap),
            "src_wr": _wrap16(src_p, ecap),
            "rels_wr": _wrap16(rels_p, ecap),
            "h0": h0,
            "hown0": hown,
            "ident_b": np.eye(P).astype(BF16_NP),
            "rq": np.asarray(inputs["r_query_embed"])[b].reshape(D, 1).astype(BF16_NP),
            "rel_bf": rel_bf,
            "msg_W": np.asarray(inputs["msg_W"]).astype(BF16_NP),
            "msg_b_col": np.ascontiguousarray(np.asarray(inputs["msg_b"]).T).astype(np.float32),
            "upd_W": np.asarray(inputs["upd_W"]).astype(BF16_NP),
            "upd_b_row": np.asarray(inputs["upd_b"]).reshape(L, D).astype(BF16_NP),
            "ln_g_row": np.asarray(inputs["ln_g"]).reshape(1, D).astype(np.float32),
            "ln_b_row": np.asarray(inputs["ln_b"]).reshape(1, D).astype(np.float32),
            "attbeta4": np.ascontiguousarray(np.concatenate(
                [np.asarray(inputs["att_W"])[:, P:2 * P, 0].T,     # a2_k [D,3]
                 np.asarray(inputs["beta_W"]).reshape(D, 1)], axis=1)).astype(BF16_NP),
            "a1": np.ascontiguousarray(
                np.asarray(inputs["att_W"])[:, 0:P, 0].T).astype(BF16_NP),   # [D,3]
            "aq6": np.ascontiguousarray(np.concatenate(
                [np.asarray(inputs["att_W"])[:, 2 * P:3 * P, 0].T,  # a3_k [D,3]
                 np.asarray(inputs["beta_W"]).reshape(D, 1),
                 np.zeros((D, 2), np.float32)], axis=1)).astype(BF16_NP),
            "sc_bias": np.concatenate([
                np.asarray(inputs["att_b"]).reshape(-1),       # 3
                np.asarray(inputs["beta_b"]).reshape(-1),      # 1
                np.asarray(inputs["den_b2"]).reshape(-1),      # 1
                np.zeros(1, np.float32),
            ]).reshape(1, 6).astype(BF16_NP),
            "den_W1": np.asarray(inputs["den_W1"]).astype(BF16_NP),
            "den_b1_row": np.asarray(inputs["den_b1"]).reshape(1, D).astype(BF16_NP),
            "den_W2": np.asarray(inputs["den_W2"]).astype(BF16_NP),
        }
        in_maps.append(m)
    return in_maps, ecap


# ----------------------------------------------------------------------------
# Device program
# ----------------------------------------------------------------------------

def build_program(ecap, reps=1, ablate=()):
    ab = set(ablate)
    C = ecap // 128          # chunks
    SLAB = 3072
    NSLAB = ecap // SLAB     # slabs (6 tiles each)
    ST = SLAB // 512         # tiles per slab (6)
    SC = SLAB // 128         # chunks per slab (24)
    CPB = (ecap // NBLK) // 128  # chunks per node block

    nc = bacc.Bacc("TRN2", num_devices=8, debug=False, num_swdge_queues=2)

    dp = nc.declare_dram_parameter
    conf_fm_d = dp("conf_fm", [D, ecap], BF16, isOutput=False)
    onehot_pm = dp("onehot_pm", [P, ecap], BF16, isOutput=False)
    srcz_row_d = dp("srcz_row", [ecap], BF16, isOutput=False)
    scores_ch_d = dp("scores_ch", [P, C], F32, isOutput=False)
    em_ch_d = dp("em_ch", [P, C], F32, isOutput=False)
    cm_ch_d = dp("cm_ch", [P, C], F32, isOutput=False)
    src_wr_d = dp("src_wr", [128, ecap // 16], I16, isOutput=False)
    rels_wr_d = dp("rels_wr", [128, ecap // 16], I16, isOutput=False)
    h0_d = dp("h0", [N, D], BF16, isOutput=False)
    hown0_d = dp("hown0", [P, NBLK * 128], F32, isOutput=False)
    ident_b_d = dp("ident_b", [P, P], BF16, isOutput=False)
    rq_d = dp("rq", [D, 1], BF16, isOutput=False)
    rel_bf_d = dp("rel_bf", [NRELP, D], BF16, isOutput=False)
    msg_W_d = dp("msg_W", [L, 5 * D, D], BF16, isOutput=False)
    msg_b_col_d = dp("msg_b_col", [D, L], F32, isOutput=False)
    upd_W_d = dp("upd_W", [L, D, D], BF16, isOutput=False)
    upd_b_row_d = dp("upd_b_row", [L, D], BF16, isOutput=False)
    ln_g_row_d = dp("ln_g_row", [1, D], F32, isOutput=False)
    ln_b_row_d = dp("ln_b_row", [1, D], F32, isOutput=False)
    attbeta4_d = dp("attbeta4", [D, 4], BF16, isOutput=False)
    a1_d = dp("a1", [D, L], BF16, isOutput=False)
    aq6_d = dp("aq6", [D, 6], BF16, isOutput=False)
    sc_bias_d = dp("sc_bias", [1, 6], BF16, isOutput=False)
    den_W1_d = dp("den_W1", [3 * D, D], BF16, isOutput=False)
    den_b1_row_d = dp("den_b1_row", [1, D], BF16, isOutput=False)
    den_W2_d = dp("den_W2", [D, 1], BF16, isOutput=False)
    out_d = dp("out", [L, D], F32, isOutput=True)

    # DRAM scratch
    hhalf = nc.dram_tensor("hhalf", [NHALF, D], BF16)
    hfull = [nc.dram_tensor(f"hfull{i}", [N, D], BF16) for i in range(2)]
    hhalf_pm = [nc.dram_tensor(f"hhalf_pm{i}", [P, NBLK * P], BF16) for i in range(2)]
    hfull2 = [nc.dram_tensor(f"hfull2_{i}", [2 * P, NBLK * P], BF16) for i in range(2)]

    AF = mybir.ActivationFunctionType
    ALU = mybir.AluOpType

    with tile.TileContext(nc) as tc:
        for _rep in range(reps):
            with (
                tc.tile_pool(name=f"res{_rep}", bufs=1) as res,
                tc.tile_pool(name=f"wgt{_rep}", bufs=1) as wgt,
            ):
                # ---------------- persistent SBUF ----------------
                hr_fm = res.tile([P, ecap], BF16)
                conf_fm = res.tile([P, ecap], BF16)
                oh_fm = res.tile([P, ecap], BF16)
                s_ch = res.tile([P, C], F32)
                attrelp = res.tile([P, L, C], F32)
                att_ch = res.tile([P, C], F32)
                w_ch = res.tile([P, C], F32)
                exab_ch = res.tile([P, C], BF16)
                em_ch = res.tile([P, C], F32)
                src_wr = res.tile([128, ecap // 16], I16)
                rel_sb = res.tile([P, NRELP // P, D], BF16)   # 2KB/part
                h_sb = res.tile([P, N // P, D], BF16)         # 4KB/part
                h_tiles = [res.tile([P, NBLK, 128], F32, name=f"h_t{i}", tag=f"h_t{i}")
                           for i in range(2)]

                # ---------------- weights in SBUF ----------------
                msgW = wgt.tile([P, L, 5, D], BF16)
                denW = wgt.tile([P, 3, D], BF16)      # A, B, C blocks of den_W1
                updW = wgt.tile([P, L, D], BF16)
                a1 = wgt.tile([P, L], BF16)
                attbeta4 = wgt.tile([P, 4], BF16)     # a2_0..2, beta_W
                aq6 = wgt.tile([P, 6], BF16)          # a3_0..2, beta_W, 0, 0
                denW2 = wgt.tile([P, 1], BF16)
                msgb = wgt.tile([P, L], F32)
                w3sum = wgt.tile([1, L, D], BF16)
                updb_row = wgt.tile([1, L, D], BF16)
                denb1_row = wgt.tile([1, D], BF16)
                rq_bf = wgt.tile([P, 1], BF16)
                scb_bf = wgt.tile([1, 6], BF16)
                ident_b = wgt.tile([P, P], BF16)
                ones_col = wgt.tile([P, 1], BF16)
                ones_r1b = wgt.tile([1, P], BF16)
                ones_r1f = wgt.tile([1, P], F32)
                ones11 = wgt.tile([1, 1], BF16)
                eps_col = wgt.tile([P, 1], F32)
                g_rep = wgt.tile([P, P], F32)
                b_rep = wgt.tile([P, P], F32)
                rep6 = wgt.tile([P, 6], F32)
                den_bias = wgt.tile([P, 1], F32)
                row6_bf = wgt.tile([1, 6], BF16)

                gp, sy, ve, sc, te = nc.gpsimd, nc.sync, nc.vector, nc.scalar, nc.tensor

                # ---------------- step 0: load + cast weights ----------------
                sy.dma_start(msgW[:], msg_W_d[:].rearrange("k (t i) o -> i k t o", i=P))
                sy.dma_start(denW[:], den_W1_d[:].rearrange("(t i) o -> i t o", i=P))
                sy.dma_start(updW[:], upd_W_d[:].rearrange("k i o -> i k o"))
                sy.dma_start(a1[:], a1_d[:])
                sy.dma_start(attbeta4[:], attbeta4_d[:])
                sy.dma_start(aq6[:], aq6_d[:])
                sy.dma_start(denW2[:], den_W2_d[:])
                sy.dma_start(msgb[:], msg_b_col_d[:])
                sy.dma_start(updb_row[:], upd_b_row_d[:].rearrange("k d -> () k d"))
                sy.dma_start(denb1_row[:], den_b1_row_d[:])
                sy.dma_start(rq_bf[:], rq_d[:])
                sy.dma_start(scb_bf[:], sc_bias_d[:])
                sy.dma_start(ident_b[:], ident_b_d[:])
                ve.memset(ones_col[:], 1.0)
                ve.memset(ones_r1b[:], 1.0)
                ve.memset(ones_r1f[:], 1.0)
                ve.memset(ones11[:], 1.0)
                ve.memset(eps_col[:], 1e-5)
                sy.dma_start(em_ch[:], em_ch_d[:])
                sy.dma_start(src_wr[:], src_wr_d[:])
                sy.dma_start(h_tiles[0][:].rearrange("p b d -> p (b d)"), hown0_d[:])
                # SBUF-resident gather sources
                # gather-source layout: token i lives at [partition i%128, slot i//128]
                sy.dma_start(rel_sb[:], rel_bf_d[:].rearrange("(a p) d -> p a d", p=P))
                sy.dma_start(h_sb[:], h0_d[:].rearrange("(a p) d -> p a d", p=P))

                with tc.tile_pool(name=f"prep_ps{_rep}", bufs=1, space="PSUM") as pps:
                    # w3sum_k = ones^T @ W3_k
                    w3ps = pps.tile([1, L, D], F32)
                    for k in range(L):
                        te.matmul(w3ps[:, k, :], ones_col[:], msgW[:, k, 2, :])
                    sc.copy(w3sum[:], w3ps[:])

                    # row6 = rq^T @ [a3_0,a3_1,a3_2,beta_W,0,0] + sc_bias
                    r6ps = pps.tile([1, 6], F32)
                    te.matmul(r6ps[:], rq_bf[:], aq6[:], start=True, stop=False)
                    te.matmul(r6ps[:], ones11[:], scb_bf[:], start=False, stop=True)
                    sc.copy(row6_bf[:], r6ps[:])

                    # rep6 = ones ⊗ row6 ; den_bias = denB^T rq + den_b1
                    rp6 = pps.tile([P, 6], F32)
                    te.matmul(rp6[:], ones_r1b[:], row6_bf[:])
                    ve.tensor_copy(rep6[:], rp6[:])

                    dbp = pps.tile([P, 1], F32)
                    te.matmul(dbp[:], denW[:, 1, :], rq_bf[:], start=True, stop=False)
                    te.matmul(dbp[:], denb1_row[:], ones11[:], start=False, stop=True)
                    ve.tensor_copy(den_bias[:], dbp[:])

                    # g_rep / b_rep (fp32 broadcast matmuls)
                    lng = wgt.tile([1, D], F32, name="lng_row")
                    lnb = wgt.tile([1, D], F32, name="lnb_row")
                    sy.dma_start(lng[:], ln_g_row_d[:])
                    sy.dma_start(lnb[:], ln_b_row_d[:])
                    grp = pps.tile([P, D], F32)
                    te.matmul(grp[:], ones_r1f[:], lng[:])
                    ve.tensor_copy(g_rep[:], grp[:])
                    brp = pps.tile([P, D], F32)
                    te.matmul(brp[:], ones_r1f[:], lnb[:])
                    ve.tensor_copy(b_rep[:], brp[:])

                # ---------------- phase A: h_r gather, den gate, att_rel ------
                with (
                    tc.tile_pool(name=f"pA3{_rep}", bufs=3) as pA3,
                    tc.tile_pool(name=f"pA_ps{_rep}", bufs=2, space="PSUM") as pAps,
                    tc.tile_pool(name=f"chA{_rep}", bufs=1) as chA,
                ):
                    rels_wr = chA.tile([128, ecap // 16], I16)
                    sy.dma_start(rels_wr[:], rels_wr_d[:])
                    scores_ch = chA.tile([P, C], F32)
                    cm_ch = chA.tile([P, C], F32)
                    betarel_ch = chA.tile([P, C], F32)
                    denlin_ch = chA.tile([P, C], F32)
                    sy.dma_start(scores_ch[:], scores_ch_d[:])
                    sy.dma_start(cm_ch[:], cm_ch_d[:])

                    for s in range(NSLAB):
                        lo = s * SLAB
                        sy.dma_start(conf_fm[:, lo:lo + SLAB], conf_fm_d[:, lo:lo + SLAB])
                        sy.dma_start(oh_fm[:, lo:lo + SLAB], onehot_pm[:, lo:lo + SLAB])
                        if "gather" in ab:
                            gp.dma_start(hr_fm[:, lo:lo + SLAB], onehot_pm[:, lo:lo + SLAB])
                        elif GATHER_SRC == "sbuf":
                            gp.dma_gather(
                                hr_fm[:, lo:lo + SLAB].rearrange("p (o e) -> p o e", o=1),
                                rel_sb[:].rearrange("p a d -> p (a d)"),
                                rels_wr[:, lo // 16:(lo + SLAB) // 16],
                                SLAB, SLAB, D, transpose=True, single_packet=False,
                                queue_num=(s % 2) if GQ == 2 else GQ,
                                sbuf_tokens_per_rank=P,
                                sbuf_free_dim_per_rank=D * 2,
                            )
                        else:
                            gp.dma_gather(
                                hr_fm[:, lo:lo + SLAB].rearrange("p (o e) -> p o e", o=1),
                                rel_bf_d[:],
                                rels_wr[:, lo // 16:(lo + SLAB) // 16],
                                SLAB, SLAB, D, transpose=True, single_packet=False,
                                queue_num=(s % 2) if GQ == 2 else GQ,
                            )

                    for s in range(NSLAB):
                        r5_ps = pAps.tile([P, SC, 5], F32, tag="r5ps")
                        hids = []
                        for tt in range(ST):
                            t = s * ST + tt
                            e0 = t * 512
                            dps = pAps.tile([P, 512], F32, tag="denps")
                            te.matmul(dps[:], denW[:, 0, :], hr_fm[:, e0:e0 + 512],
                                      start=True, stop=False)
                            te.matmul(dps[:], denW[:, 2, :], conf_fm[:, e0:e0 + 512],
                                      start=False, stop=True)
                            hid = pA3.tile([P, 512], BF16, tag="hid")
                            sc.activation(hid[:], dps[:], AF.Relu, bias=den_bias[:])
                            hids.append(hid)
                            # att_rel / beta projections, chunk layout
                            for j in range(4):
                                cc = 4 * tt + j
                                te.matmul(r5_ps[:, cc, 0:4],
                                          hr_fm[:, e0 + 128 * j:e0 + 128 * j + 128],
                                          attbeta4[:])
                            # den hidden -> den_lin, one tile behind (hides relu)
                            if tt > 0:
                                for j in range(4):
                                    cc = 4 * (tt - 1) + j
                                    te.matmul(r5_ps[:, cc, 4:5],
                                              hids[tt - 1][:, 128 * j:128 * j + 128],
                                              denW2[:])
                        for j in range(4):
                            cc = 4 * (ST - 1) + j
                            te.matmul(r5_ps[:, cc, 4:5],
                                      hids[ST - 1][:, 128 * j:128 * j + 128],
                                      denW2[:])

                        c0 = s * SC
                        for k in range(L):
                            ve.tensor_copy(attrelp[:, k, c0:c0 + SC], r5_ps[:, :, k])
                        ve.tensor_copy(betarel_ch[:, c0:c0 + SC], r5_ps[:, :, 3])
                        ve.tensor_copy(denlin_ch[:, c0:c0 + SC], r5_ps[:, :, 4])

                    # chunk-layout gate math
                    beta_t = chA.tile([P, C], F32)
                    sc.activation(beta_t[:], betarel_ch[:], AF.Sigmoid, bias=rep6[:, 3:4])
                    tmp_t = chA.tile([P, C], F32)
                    ve.tensor_tensor(tmp_t[:], scores_ch[:], beta_t[:], ALU.subtract)
                    gk_t = chA.tile([P, C], F32)
                    sc.activation(gk_t[:], tmp_t[:], AF.Sigmoid, scale=1.0 / TAU)
                    ve.tensor_scalar(gk_t[:], gk_t[:], -0.5, None, ALU.add)
                    ve.tensor_tensor(gk_t[:], cm_ch[:], gk_t[:], ALU.mult)
                    ve.tensor_scalar(gk_t[:], gk_t[:], 0.5, None, ALU.add)   # gate
                    den_t = chA.tile([P, C], F32)
                    sc.activation(den_t[:], denlin_ch[:], AF.Sigmoid, bias=rep6[:, 4:5])
                    ve.tensor_tensor(s_ch[:], gk_t[:], den_t[:], ALU.mult)
                    ve.tensor_tensor(s_ch[:], s_ch[:], em_ch[:], ALU.mult)
                    for k in range(L):
                        ve.tensor_scalar(attrelp[:, k, :], attrelp[:, k, :],
                                         rep6[:, k:k + 1], None, ALU.add)

                # ---------------- phase B: layers ----------------
                for k in range(L):
                    with tc.tile_pool(name=f"ups{k}_{_rep}", bufs=1, space="PSUM") as upool:
                      # fused scatter accumulators: 8 node blocks x 129 cols
                      # (128 features + exab sum), packed 3/3/2 blocks per bank
                      us = [upool.tile([P, 512], F32, name=f"us{k}_{i}")
                            for i in range(3)]
                      with (
                        tc.tile_pool(name=f"lps{k}_{_rep}", bufs=1, space="PSUM") as lpool,
                        tc.tile_pool(name=f"sl{k}_{_rep}", bufs=2) as slp,
                        tc.tile_pool(name=f"tp{k}_{_rep}", bufs=3) as tpp,
                        tc.tile_pool(name=f"rm{k}_{_rep}", bufs=4) as rmp,
                        tc.tile_pool(name=f"tb{k}_{_rep}", bufs=8) as tbp,
                      ):
                        # transpose rhs: [identity | a1_k] -> one matmul gives
                        # both the chunk transpose and the att column
                        augr = slp.tile([P, 129], BF16, tag="augr", bufs=1)
                        ve.tensor_copy(augr[:, 0:128], ident_b[:])
                        ve.tensor_copy(augr[:, 128:129], a1[:, k:k + 1])

                        trpsbs = {}

                        def emit_aug(s, tt):
                            # transpose+att for tile tt of slab s: one 2-bank
                            # PSUM tile holds all 4 chunk regions (2 per bank)
                            trpsb = tbp.tile([P, 4, P], BF16, tag="trpsb")
                            c0 = s * SC
                            rmsg = trpsbs[(s, tt, "rmsg")]
                            a_ps = lpool.tile([P, 2, 512], F32, tag="augps", bufs=1)
                            for jj in range(4):
                                h, q = jj // 2, jj % 2
                                te.matmul(a_ps[:, h, 129 * q:129 * q + 129],
                                          rmsg[:, 128 * jj:128 * jj + 128],
                                          augr[:])
                            av = a_ps[:, :, 0:258].rearrange("p h (q y) -> p h q y", q=2)
                            sc.copy(trpsb[:].rearrange("p (h q) y -> p h q y", h=2),
                                    av[:, :, :, 0:128])
                            ve.tensor_copy(
                                att_ch[:, c0 + 4 * tt:c0 + 4 * tt + 4]
                                .rearrange("p (h q) -> p h q", h=2),
                                av[:, :, :, 128])
                            trpsbs[(s, tt)] = trpsb

                        def emit_scatter(s):
                            # slab chunk math + weighted scatter for slab s
                            c0 = s * SC
                            ve.tensor_tensor(att_ch[:, c0:c0 + SC], att_ch[:, c0:c0 + SC],
                                             attrelp[:, k, c0:c0 + SC], ALU.add)
                            lr_t = tpp.tile([P, SC], F32, tag="lrt")
                            ve.tensor_scalar(lr_t[:], att_ch[:, c0:c0 + SC], 0.01, None,
                                             ALU.mult)
                            ve.tensor_tensor(att_ch[:, c0:c0 + SC], att_ch[:, c0:c0 + SC],
                                             lr_t[:], ALU.max)
                            sc.activation(att_ch[:, c0:c0 + SC], att_ch[:, c0:c0 + SC],
                                          AF.Exp)
                            ve.tensor_tensor(att_ch[:, c0:c0 + SC], att_ch[:, c0:c0 + SC],
                                             em_ch[:, c0:c0 + SC], ALU.mult)   # em*exp(att)
                            ve.tensor_copy(exab_ch[:, c0:c0 + SC], att_ch[:, c0:c0 + SC])
                            ve.tensor_tensor(w_ch[:, c0:c0 + SC], att_ch[:, c0:c0 + SC],
                                             s_ch[:, c0:c0 + SC], ALU.mult)
                            if "scatter" in ab:
                                return
                            for tt in range(ST):
                                t = s * ST + tt
                                wm = tpp.tile([P, 4, 129], BF16, tag="wm")
                                ve.tensor_tensor(
                                    wm[:, :, 0:128], trpsbs.pop((s, tt))[:],
                                    w_ch[:, 4 * t:4 * t + 4].broadcast_to([P, 4, P]),
                                    ALU.mult,
                                )
                                ve.tensor_copy(wm[:, :, 128], exab_ch[:, 4 * t:4 * t + 4])
                                e0 = t * 512
                                for j in range(4):
                                    c = 4 * t + j
                                    blk = c // CPB
                                    bi = blk // 3
                                    off = (blk - 3 * bi) * 129
                                    ust = c in (0, 3 * CPB, 6 * CPB)
                                    usp = c in (3 * CPB - 1, 6 * CPB - 1, 8 * CPB - 1)
                                    te.matmul(us[bi][:, off:off + 129],
                                              oh_fm[:, e0 + 128 * j:e0 + 128 * j + 128],
                                              wm[:, j, :],
                                              start=ust, stop=usp)

                        for s in range(NSLAB):
                            lo = s * SLAB
                            hsrc_sl = slp.tile([P, SLAB], BF16, tag="hsrc", bufs=3)
                            if "gather" in ab:
                                gp.dma_start(hsrc_sl[:], onehot_pm[:, lo:lo + SLAB])
                            elif GATHER_SRC == "sbuf":
                                gp.dma_gather(
                                    hsrc_sl[:].rearrange("p (o e) -> p o e", o=1),
                                    h_sb[:].rearrange("p a d -> p (a d)"),
                                    src_wr[:, lo // 16:(lo + SLAB) // 16],
                                    SLAB, SLAB, D, transpose=True, single_packet=False,
                                    queue_num=(s % 2) if GQ == 2 else GQ,
                                    sbuf_tokens_per_rank=P,
                                    sbuf_free_dim_per_rank=D * 2,
                                )
                            else:
                                h_read = h0_d if k == 0 else hfull[(k - 1) % 2]
                                gp.dma_gather(
                                    hsrc_sl[:].rearrange("p (o e) -> p o e", o=1),
                                    h_read[:],
                                    src_wr[:, lo // 16:(lo + SLAB) // 16],
                                    SLAB, SLAB, D, transpose=True, single_packet=False,
                                    queue_num=(s % 2) if GQ == 2 else GQ,
                                )

                            srcz_sl = slp.tile([1, SLAB], BF16, tag="srcz")
                            sy.dma_start(srcz_sl[:],
                                         srcz_row_d[lo:lo + SLAB].rearrange("e -> () e"))
                            if s > 0:
                                emit_scatter(s - 1)
                            for tt in range(ST):
                                t = s * ST + tt
                                e0, f0 = t * 512, tt * 512
                                prod = tpp.tile([P, 512], BF16, tag="prod")
                                ve.tensor_tensor(prod[:], hsrc_sl[:, f0:f0 + 512],
                                                 hr_fm[:, e0:e0 + 512], ALU.mult)
                                mps = lpool.tile([P, 512], F32, tag="msgps", bufs=2)
                                te.matmul(mps[:], msgW[:, k, 0, :], prod[:],
                                          start=True, stop=False)
                                te.matmul(mps[:], msgW[:, k, 1, :], hsrc_sl[:, f0:f0 + 512],
                                          start=False, stop=False)
                                te.matmul(mps[:], msgW[:, k, 3, :], hr_fm[:, e0:e0 + 512],
                                          start=False, stop=False)
                                te.matmul(mps[:], msgW[:, k, 4, :], conf_fm[:, e0:e0 + 512],
                                          start=False, stop=False)
                                te.matmul(mps[:], w3sum[:, k, :],
                                          srcz_sl[:, f0:f0 + 512],
                                          start=False, stop=True)
                                rmsg = rmp.tile([P, 512], BF16, tag="rmsg")
                                sc.activation(rmsg[:], mps[:], AF.Relu, bias=msgb[:, k:k + 1])
                                trpsbs[(s, tt, "rmsg")] = rmsg
                                # transpose+att one tile behind (hides relu latency)
                                if tt > 0:
                                    emit_aug(s, tt - 1)
                            emit_aug(s, ST - 1)
                        emit_scatter(NSLAB - 1)

                      if "scatter" in ab:
                          for bi in range(3):
                              te.matmul(us[bi][:, 0:128], ident_b[:], ident_b[:],
                                        start=True, stop=True)
                      # ---------------- layer tail ----------------
                      if True:
                        with (
                            tc.tile_pool(name=f"tl{k}_{_rep}", bufs=1) as tlp,
                            tc.tile_pool(name=f"tlps{k}_{_rep}", bufs=1, space="PSUM") as tlps,
                        ):
                            nb_per = (3, 3, 2)
                            us_r = [us[i][:, 0:129 * nb_per[i]]
                                    .rearrange("p (b x) -> p b x", x=129)
                                    for i in range(3)]
                            sm_s = tlp.tile([P, NBLK], F32)
                            rsm = tlp.tile([P, NBLK], F32)
                            aggr = tlp.tile([P, NBLK, P], BF16)
                            b0 = 0
                            for i in range(3):
                                nb = nb_per[i]
                                ve.tensor_scalar(sm_s[:, b0:b0 + nb],
                                                 us_r[i][:, :, 128], 1e-8, None, ALU.add)
                                b0 += nb
                            ve.reciprocal(rsm[:], sm_s[:])
                            b0 = 0
                            for i in range(3):
                                nb = nb_per[i]
                                ve.tensor_tensor(aggr[:, b0:b0 + nb, :],
                                                 us_r[i][:, :, 0:128],
                                                 rsm[:, b0:b0 + nb].broadcast_to([P, nb, P]),
                                                 ALU.mult)
                                b0 += nb
                            aggrT = tlp.tile([P, NBLK, P], BF16)
                            trp2 = tlps.tile([P, NBLK, P], BF16, bufs=1, tag="tr2")
                            for bb in range(NBLK):
                                te.transpose(trp2[:, bb, :], aggr[:, bb, :], ident_b[:])
                            ve.tensor_copy(aggrT[:], trp2[:])

                            hb_ps = tlps.tile([P, NBLK, P], F32, tag="hb")
                            for bb in range(NBLK):
                                te.matmul(hb_ps[:, bb, :], aggrT[:, bb, :], updW[:, k, :],
                                          start=True, stop=False)
                                te.matmul(hb_ps[:, bb, :], ones_r1b[:], updb_row[:, k, :],
                                          start=False, stop=True)

                            hs = tlp.tile([P, NBLK, P], F32)
                            ve.tensor_tensor(hs[:], hb_ps[:], h_tiles[k % 2][:], ALU.add)
                            mu = tlp.tile([P, NBLK], F32)
                            ve.tensor_reduce(mu[:], hs[:], mybir.AxisListType.X, ALU.add)
                            ve.tensor_scalar(mu[:], mu[:], 1.0 / P, None, ALU.mult)
                            xc = tlp.tile([P, NBLK, P], F32)
                            ve.tensor_tensor(xc[:], hs[:], mu[:].broadcast_to([P, NBLK, P]),
                                             ALU.subtract)
                            sq = tlp.tile([P, NBLK, P], F32)
                            sc.activation(sq[:], xc[:], AF.Square)
                            var = tlp.tile([P, NBLK], F32)
                            ve.tensor_reduce(var[:], sq[:], mybir.AxisListType.X, ALU.add)
                            ve.tensor_scalar(var[:], var[:], 1.0 / P, None, ALU.mult)
                            sd = tlp.tile([P, NBLK], F32)
                            sc.activation(sd[:], var[:], AF.Sqrt, bias=eps_col[:])
                            rsd = tlp.tile([P, NBLK], F32)
                            ve.reciprocal(rsd[:], sd[:])
                            hn = h_tiles[(k + 1) % 2]
                            ve.tensor_tensor(hn[:], xc[:], rsd[:].broadcast_to([P, NBLK, P]),
                                             ALU.mult)
                            ve.tensor_tensor(hn[:], hn[:],
                                             g_rep[:].rearrange("p d -> p () d").broadcast_to([P, NBLK, P]),
                                             ALU.mult)
                            ve.tensor_tensor(hn[:], hn[:],
                                             b_rep[:].rearrange("p d -> p () d").broadcast_to([P, NBLK, P]),
                                             ALU.add)

                            sy.dma_start(out_d[k:k + 1, :], hn[0:1, 0, :])

                            if k < L - 1:
                                hstage = tlp.tile([P, NBLK, P], BF16)
                                ve.tensor_copy(hstage[:], hn[:])
                                if GATHER_SRC == "sbuf":
                                    # permuted staging: node b*128+p stays at
                                    # partition p, so the post-collective DMA
                                    # back into SBUF is fully sequential
                                    sy.dma_start(
                                        hhalf_pm[k % 2][:],
                                        hstage[:].rearrange("p b d -> p (b d)"),
                                    )
                                    if "cc" in ab:
                                        gp.dma_start(hfull2[k % 2][0:P, :],
                                                     hhalf_pm[k % 2][:])
                                    else:
                                        gp.collective_compute(
                                            "AllGather",
                                            ALU.bypass,
                                            replica_groups=[[0, 1], [2, 3], [4, 5], [6, 7]],
                                            ins=[hhalf_pm[k % 2][:].opt()],
                                            outs=[hfull2[k % 2][:].opt()],
                                        )
                                    sy.dma_start(
                                        h_sb[:].rearrange("p (c a) d -> p c (a d)", c=2),
                                        hfull2[k % 2][:].rearrange("(c p) x -> p c x", p=P),
                                    )
                                else:
                                    sy.dma_start(
                                        hhalf[:].rearrange("(b p) d -> p b d", p=P),
                                        hstage[:],
                                    )
                                    if "cc" in ab:
                                        gp.dma_start(hfull[k % 2][0:NHALF, :], hhalf[:])
                                    else:
                                        gp.collective_compute(
                                            "AllGather",
                                            ALU.bypass,
                                            replica_groups=[[0, 1], [2, 3], [4, 5], [6, 7]],
                                            ins=[hhalf[:].opt()],
                                            outs=[hfull[k % 2][:].opt()],
                                        )


    nc.compile()
    return nc


_PROGRAM_CACHE = {}


def _get_program(ecap):
    if ecap not in _PROGRAM_CACHE:
        _PROGRAM_CACHE[ecap] = build_program(ecap)
    return _PROGRAM_CACHE[ecap]


def kernel(**inputs):
    in_maps, ecap = prepare_core_inputs(inputs)
    nc = _get_program(ecap)
    res = run_bass_kernel_spmd(nc, in_maps, list(range(8)))
    outs = np.stack([np.asarray(res.results[2 * b]["out"]) for b in range(B)], axis=0)
    return outs.astype(np.float32)
